# revision 1
# baseline (speedup 1.0000x reference)
"""MultiHeadRelativeAttention (Transformer-XL style) on 8 Trainium2 NeuronCores.

Sharding: batch*head-half per core (core c -> batch c//2, heads half c%2).
Each core computes its batch's 8 heads fully plus a partial out-projection
over its 512 input channels; the host sums the two partials per batch.

Key device-side trick: the Transformer-XL relative shift
    pos_shift[q, k] = pos[q, 1023 + k - q]
is one SBUF->SBUF DMA per (head, q-tile) with a hand-built access pattern
[[BAND-1, 128], [1, 1024]] offset 127 over a [128, BAND] band tile: the DMA
descriptor generator decomposes the flat element index into
(partition, byte) per partition, giving each partition q a start offset of
127 - q elements.
"""
import sys

sys.path.insert(0, '/opt/trn_rl_repo')

import numpy as np
import ml_dtypes

S = 1024          # seq len (query == key)
B = 4             # batch
E = 1024          # embed dim
H = 16            # total heads
D = 64            # head dim
HH = 8            # heads per core
PAIRS = HH // 2
MT = 2048         # padded positional length (2*S-1 = 2047 valid)
BAND = S + 128    # per-q-tile band width in m
QT = S // 128     # q tiles
KC = S // 128     # k chunks of 128
SCALING = D ** -0.5
N_CORES = 8

_cache = {}


def _build():
    import concourse.bass as bass
    from concourse import bacc
    import concourse.mybir as mybir
    from concourse.tile import TileContext

    bf16 = mybir.dt.bfloat16
    f32 = mybir.dt.float32
    f32r = mybir.dt.float32r
    Exp = mybir.ActivationFunctionType.Exp

    nc = bacc.Bacc("TRN2", debug=False, num_devices=N_CORES)

    def din(name, shape, dt=bf16):
        return nc.dram_tensor(name, shape, dt, kind='ExternalInput')

    qTin = din('qTin', [E, S])
    kTin = din('kTin', [E, S])
    vTin = din('vTin', [E, S])
    peT = din('peT', [E, MT])
    wqT = din('wqT', [E, 512])
    wkT = din('wkT', [E, 512])
    wvT = din('wvT', [E, 512])
    wpT = din('wpT', [E, 512])
    woT = din('woT', [512, E], f32r)
    cbv = din('cb', [512, 1], f32)
    pbv = din('pb', [512, 1], f32)
    ident = din('ident', [128, 128])
    outT = nc.dram_tensor('outT', [E, S], f32, kind='ExternalOutput')

    EC = E // 128  # e chunks

    with TileContext(nc) as tc:
        with tc.tile_pool(name='persist', bufs=1) as PERS, \
             tc.tile_pool(name='stage', bufs=10) as STG:

            # ---- persistent tiles ----
            qcT = [PERS.tile([128, S], bf16, name=f'qcT{p}', tag=f'qcT{p}') for p in range(PAIRS)]
            qpT = [PERS.tile([128, S], bf16, name=f'qpT{p}', tag=f'qpT{p}') for p in range(PAIRS)]
            kTt = [PERS.tile([128, S], bf16, name=f'kT{p}', tag=f'kT{p}') for p in range(PAIRS)]
            kpT = [PERS.tile([128, MT], bf16, name=f'kpT{p}', tag=f'kpT{p}') for p in range(PAIRS)]
            vS = [PERS.tile([128, 512], bf16, name=f'vS{k}', tag=f'vS{k}') for k in range(KC)]
            woS = [PERS.tile([128, E], f32r, name=f'woS{p}', tag=f'woS{p}') for p in range(PAIRS)]
            oT = [[PERS.tile([128, 512], f32r, name=f'oT{p}_{sc}', tag=f'oT{p}_{sc}') for sc in range(2)]
                  for p in range(PAIRS)]
            idS = PERS.tile([128, 128], bf16, name='idS', tag='idS')
            cbS = [PERS.tile([128, 1], f32, name=f'cbS{p}', tag=f'cbS{p}') for p in range(PAIRS)]
            pbS = [PERS.tile([128, 1], f32, name=f'pbS{p}', tag=f'pbS{p}') for p in range(PAIRS)]

            nc.sync.dma_start(idS[:], ident.ap())
            for p in range(PAIRS):
                nc.sync.dma_start(cbS[p][:], cbv.ap()[p * 128:(p + 1) * 128, :])
                nc.sync.dma_start(pbS[p][:], pbv.ap()[p * 128:(p + 1) * 128, :])
                nc.sync.dma_start(woS[p][:], woT.ap()[p * 128:(p + 1) * 128, :])

            # ---- projections ----
            with tc.tile_pool(name='pp', bufs=2, space='PSUM') as PP:
                # q projection -> qcT (+content bias) and qpT (+pos bias)
                xin = [STG.tile([128, S], bf16, name='xin', tag='xin') for _ in range(EC)]
                win = [STG.tile([128, 512], bf16, name='win', tag='win') for _ in range(EC)]
                for ec in range(EC):
                    nc.sync.dma_start(xin[ec][:], qTin.ap()[ec * 128:(ec + 1) * 128, :])
                    nc.sync.dma_start(win[ec][:], wqT.ap()[ec * 128:(ec + 1) * 128, :])
                for p in range(PAIRS):
                    ps = PP.tile([128, S], f32, name='qps', tag='qps')
                    for c in range(2):
                        for ec in range(EC):
                            nc.tensor.matmul(
                                ps[:, c * 512:(c + 1) * 512],
                                win[ec][:, p * 128:(p + 1) * 128],
                                xin[ec][:, c * 512:(c + 1) * 512],
                                start=(ec == 0), stop=(ec == EC - 1))
                    nc.vector.tensor_scalar_add(qcT[p][:], ps[:], cbS[p][:])
                    nc.vector.tensor_scalar_add(qpT[p][:], ps[:], pbS[p][:])

                # k projection -> kTt
                xin2 = [STG.tile([128, S], bf16, name='xin', tag='xin') for _ in range(EC)]
                win2 = [STG.tile([128, 512], bf16, name='win', tag='win') for _ in range(EC)]
                for ec in range(EC):
                    nc.sync.dma_start(xin2[ec][:], kTin.ap()[ec * 128:(ec + 1) * 128, :])
                    nc.sync.dma_start(win2[ec][:], wkT.ap()[ec * 128:(ec + 1) * 128, :])
                for p in range(PAIRS):
                    ps = PP.tile([128, S], f32, name='qps', tag='qps')
                    for c in range(2):
                        for ec in range(EC):
                            nc.tensor.matmul(
                                ps[:, c * 512:(c + 1) * 512],
                                win2[ec][:, p * 128:(p + 1) * 128],
                                xin2[ec][:, c * 512:(c + 1) * 512],
                                start=(ec == 0), stop=(ec == EC - 1))
                    nc.scalar.copy(kTt[p][:], ps[:])

                # k_pos projection -> kpT  (rhs = peT chunks)
                win3 = [STG.tile([128, 512], bf16, name='win', tag='win') for _ in range(EC)]
                for ec in range(EC):
                    nc.sync.dma_start(win3[ec][:], wpT.ap()[ec * 128:(ec + 1) * 128, :])
                for mc in range(MT // 512):
                    pein = [STG.tile([128, 512], bf16, name='pein', tag='pein') for _ in range(EC)]
                    for ec in range(EC):
                        nc.sync.dma_start(
                            pein[ec][:],
                            peT.ap()[ec * 128:(ec + 1) * 128, mc * 512:(mc + 1) * 512])
                    for p in range(PAIRS):
                        ps = PP.tile([128, 512], f32, name='sps', tag='sps')
                        for ec in range(EC):
                            nc.tensor.matmul(
                                ps[:], win3[ec][:, p * 128:(p + 1) * 128], pein[ec][:],
                                start=(ec == 0), stop=(ec == EC - 1))
                        nc.scalar.copy(kpT[p][:, mc * 512:(mc + 1) * 512], ps[:])

                # v projection -> vS[kt] = [128 k, (h,d) 512]; stationary = valueT chunk
                xin3 = [STG.tile([128, S], bf16, name='xin', tag='xin') for _ in range(EC)]
                win4 = [STG.tile([128, 512], bf16, name='win', tag='win') for _ in range(EC)]
                for ec in range(EC):
                    nc.sync.dma_start(xin3[ec][:], vTin.ap()[ec * 128:(ec + 1) * 128, :])
                    nc.sync.dma_start(win4[ec][:], wvT.ap()[ec * 128:(ec + 1) * 128, :])
                for kt in range(KC):
                    ps = PP.tile([128, 512], f32, name='sps', tag='sps')
                    for ec in range(EC):
                        nc.tensor.matmul(
                            ps[:], xin3[ec][:, kt * 128:(kt + 1) * 128], win4[ec][:],
                            start=(ec == 0), stop=(ec == EC - 1))
                    nc.scalar.copy(vS[kt][:], ps[:])

            # ---- scores + attention ----
            with tc.tile_pool(name='pP', bufs=3, space='PSUM') as PSP, \
                 tc.tile_pool(name='pC', bufs=2, space='PSUM') as PSC, \
                 tc.tile_pool(name='pO', bufs=1, space='PSUM') as PSO, \
                 tc.tile_pool(name='atp', bufs=1) as ATP, \
                 tc.tile_pool(name='scp', bufs=3) as SCP:

                for p in range(PAIRS):
                    at = [ATP.tile([128, KC, QT, 128], bf16, name=f'at{h01}', tag=f'at{h01}')
                          for h01 in range(2)]
                    for h01 in range(2):
                        rows = slice(64 * h01, 64 * h01 + 64)
                        as_tiles = []
                        for t in range(QT):
                            blo = 896 - 128 * t
                            qsl = slice(t * 128, (t + 1) * 128)
                            # pos band matmul: P[qt, j] = qp[q] . kpos[blo + j]
                            # 512-wide PSUM chunks, evicted immediately (3-deep pipe)
                            pb_t = SCP.tile([128, BAND], bf16, name='Pb', tag='Pb')
                            for ci, (c0, n) in enumerate(((0, 512), (512, 512), (1024, 128))):
                                pps = PSP.tile([128, 512], f32, name='P', tag='P')
                                nc.tensor.matmul(
                                    pps[:, :n],
                                    qpT[p][rows, qsl],
                                    kpT[p][rows, blo + c0:blo + c0 + n],
                                    start=True, stop=True)
                                if (t * 3 + ci) % 2 == 0:
                                    nc.vector.tensor_copy(pb_t[:, c0:c0 + n], pps[:, :n])
                                else:
                                    nc.scalar.copy(pb_t[:, c0:c0 + n], pps[:, :n])
                            # rel-shift: sheared SBUF->SBUF DMA (SWDGE; GpSimd DGE is idle)
                            src = pb_t[:]
                            sheared = src.__replace__(
                                ap=src.ap.__class__([[BAND - 1, 128], [1, S]]),
                                offset=127)
                            ps_t = SCP.tile([128, S], bf16, name='Ps', tag='Ps')
                            nc.gpsimd.dma_start(ps_t[:], sheared)
                            # content first (independent), then identity-add of sheared pos
                            cps = PSC.tile([128, S], f32, name='C', tag='C')
                            for c in range(2):
                                csl = slice(c * 512, (c + 1) * 512)
                                nc.tensor.matmul(
                                    cps[:, csl], qcT[p][rows, qsl], kTt[p][rows, csl],
                                    start=True, stop=False)
                            for c in range(2):
                                csl = slice(c * 512, (c + 1) * 512)
                                nc.tensor.matmul(
                                    cps[:, csl], idS[:], ps_t[:, csl],
                                    start=False, stop=True)
                            # softmax (no max-subtraction; logits are pre-scaled)
                            a_t = SCP.tile([128, S], bf16, name='A', tag='A')
                            den = SCP.tile([128, 1], f32, name='den', tag='den')
                            nc.scalar.activation(a_t[:], cps[:], Exp, accum_out=den[:])
                            rec = SCP.tile([128, 1], f32, name='rec', tag='rec')
                            nc.vector.reciprocal(rec[:], den[:])
                            as_t = SCP.tile([128, S], bf16, name='As', tag='As', bufs=10)
                            nc.vector.tensor_scalar_mul(as_t[:], a_t[:], rec[:])
                            as_tiles.append(as_t)
                        # batched xbar transposes (minimize DMA xbar-mode transitions)
                        for t in range(QT):
                            nc.sync.dma_start_transpose(at[h01][:, :, t, :], as_tiles[t])

                    # attn @ V for the pair (col-tiled heads)
                    for sc in range(2):
                        ops = PSO.tile([128, 512], f32, name='O', tag='O')
                        for kc in range(KC):
                            for h01 in range(2):
                                cb0 = (2 * p + h01) * 64
                                nc.tensor.matmul(
                                    ops[64 * h01:64 * h01 + 64, :],
                                    vS[kc][:, cb0:cb0 + 64],
                                    at[h01][:, kc, 4 * sc:4 * sc + 4, :],
                                    start=(kc == 0), stop=(kc == KC - 1))
                        nc.scalar.copy(oT[p][sc][:], ops[:])

            # ---- out projection (f32r) ----
            with tc.tile_pool(name='op', bufs=2, space='PSUM') as OPP, \
                 tc.tile_pool(name='oev', bufs=3) as OEV:
                for sc in range(2):
                    for eb in range(EC):
                        ps = OPP.tile([128, 512], f32, name='OP', tag='OP')
                        for p in range(PAIRS):
                            nc.tensor.matmul(
                                ps[:],
                                woS[p][:, eb * 128:(eb + 1) * 128].bitcast(f32r),
                                oT[p][sc][:].bitcast(f32r),
                                start=(p == 0), stop=(p == PAIRS - 1))
                        ev = OEV.tile([128, 512], f32, name='oev', tag='oev')
                        nc.vector.tensor_copy(ev[:], ps[:])
                        nc.sync.dma_start(
                            outT.ap()[eb * 128:(eb + 1) * 128, sc * 512:(sc + 1) * 512],
                            ev[:])

    nc.compile()
    return nc


def _prep_inputs(inputs):
    """Full inputs -> list of per-core input dicts (host-side shard + layout)."""
    bf = ml_dtypes.bfloat16
    q = np.asarray(inputs['query'], np.float32)
    k = np.asarray(inputs['key'], np.float32)
    v = np.asarray(inputs['value'], np.float32)
    pe = np.asarray(inputs['pe'], np.float32)
    w_q = np.asarray(inputs['w_q'], np.float32)
    w_k = np.asarray(inputs['w_k'], np.float32)
    w_v = np.asarray(inputs['w_v'], np.float32)
    w_kp = np.asarray(inputs['w_k_pos'], np.float32)
    cb = np.asarray(inputs['content_bias'], np.float32)
    pb = np.asarray(inputs['pos_bias'], np.float32)
    w_out = np.asarray(inputs['w_out'], np.float32)

    M = 2 * S - 1
    lower = pe.shape[0] // 2 - S + 1
    pe_sl = pe[lower:lower + M]                     # [2047, E]
    peT = np.zeros((E, MT), np.float32)
    peT[:, :M] = pe_sl.T

    ident = np.eye(128, dtype=bf)

    in_maps = []
    for c in range(N_CORES):
        b, half = divmod(c, 2)
        hs = half * HH
        ch = slice(hs * D, (hs + HH) * D)           # this core's 512 channels
        in_maps.append({
            'qTin': np.ascontiguousarray(q[:, b, :].T).astype(bf),
            'kTin': np.ascontiguousarray(k[:, b, :].T).astype(bf),
            'vTin': np.ascontiguousarray(v[:, b, :].T).astype(bf),
            'peT': peT.astype(bf),
            'wqT': np.ascontiguousarray((SCALING * w_q[ch, :]).T).astype(bf),
            'wkT': np.ascontiguousarray(w_k[ch, :].T).astype(bf),
            'wvT': np.ascontiguousarray(w_v[ch, :].T).astype(bf),
            'wpT': np.ascontiguousarray(w_kp[ch, :].T).astype(bf),
            'woT': np.ascontiguousarray(w_out[:, ch].T).astype(np.float32),
            'cb': (SCALING * cb[hs:hs + HH].reshape(512, 1)).astype(np.float32),
            'pb': (SCALING * pb[hs:hs + HH].reshape(512, 1)).astype(np.float32),
            'ident': ident,
        })
    return in_maps


def kernel(**inputs):
    from concourse import bass_utils

    if 'nc' not in _cache:
        _cache['nc'] = _build()
    nc = _cache['nc']

    in_maps = _prep_inputs(inputs)
    res = bass_utils.run_bass_kernel_spmd(nc, in_maps, core_ids=list(range(N_CORES)))
    _cache['last_results'] = res

    b_out = np.asarray(inputs['b_out'], np.float32)
    out = np.empty((S, B, E), np.float32)
    for b in range(B):
        acc = res.results[2 * b]['outT'] + res.results[2 * b + 1]['outT']
        out[:, b, :] = acc.T + b_out
    return out

